# revision 7
# baseline (speedup 1.0000x reference)
"""VQ codebook quantization kernel for Trainium2 (8 NeuronCores, SPMD).

Problem: z_e [4, 256, 32, 32, 32] f32, emb [1024, 256] f32.
  tokens = z_e reshaped to [B, N=32768, C=256]; for each token find the
  nearest codebook row (squared L2), output the gathered embeddings in the
  original [B, C, D, H, W] layout plus the scalar VQ loss.

Sharding: data-parallel over the 131072 tokens -> 8 shards of 16384 tokens
(core i handles batch b = i//2, token half i%2). The [1024, 256] codebook is
replicated. Each core returns its z_q shard plus per-partition loss partials;
the host reassembles the full tensor and reduces the loss scalar.

Per-core algorithm (all fp32):
  score[t, k] = z_t . e_k - |e_k|^2/2   (argmax_k score == argmin_k dist^2)
  - cross term via PE matmuls: lhsT = z slice [c,t] (natural DRAM layout),
    rhs = embT (one-time on-chip transpose of emb)
  - bias -|e_k|^2/2 precomputed broadcast across partitions (one-time),
    added during the PSUM->SBUF move (DVE)
  - argmax over k via DVE max8 + max_index
  - gather emb[idx] via indirect DMA (one row per partition), transpose the
    gathered [t, c] tile back to [c, t] on the PE
  - loss uses sum((z - e*)^2) = sum(z^2) - 2 * sum(max score): no per-element
    diff needed.  sum(z^2) via ACT Square-accumulate, sum(max) from the max8
    results.
"""

import numpy as np

import concourse.bass as bass
import concourse.mybir as mybir
import concourse.tile as tile
from concourse import bass_utils

F32 = mybir.dt.float32

B, C, DHW = 4, 256, 32 * 32 * 32
K = 1024
N_CORES = 8
TOK_PER_CORE = (B * DHW) // N_CORES  # 16384
GROUP = 1024                         # tokens per DMA group
N_GROUPS = TOK_PER_CORE // GROUP     # 16
TILES_PER_GROUP = GROUP // 128       # 8
N_TILES = N_GROUPS * TILES_PER_GROUP # 128


def _split_multi_waits(nc):
    """This container's walrus rejects more than one sync wait per
    instruction.  Hoist excess waits onto wait-only EventSemaphore carriers
    inserted immediately before the offending instruction on the same
    engine's stream (semantically identical: the engine blocks on the
    carrier first)."""
    n_carrier = 0
    for bb in nc.m.functions[0].blocks:
        insts = list(bb.instructions)
        if not any(
            i.sync_info is not None and i.sync_info.on_wait and len(i.sync_info.on_wait) > 1
            for i in insts
        ):
            continue
        newlist = []
        for ins in insts:
            si = ins.sync_info
            if si is not None and si.on_wait and len(si.on_wait) > 1:
                waits = list(si.on_wait)
                for w in waits[:-1]:
                    ev = mybir.InstEventSemaphore(
                        name=f"wsplit-{n_carrier}", ins=[], outs=[]
                    )
                    n_carrier += 1
                    ev.engine = ins.engine
                    ev.sync_info = mybir.SyncInfo(on_wait=[w], on_update=[])
                    newlist.append(ev)
                ins.sync_info = mybir.SyncInfo(
                    on_wait=[waits[-1]], on_update=list(si.on_update or [])
                )
            newlist.append(ins)
        bb.instructions = newlist


def build():
    from concourse.masks import make_identity

    nc = bass.Bass("TRN2", target_bir_lowering=False, debug=False)
    z = nc.dram_tensor("z", [C, TOK_PER_CORE], F32, kind="ExternalInput")
    emb = nc.dram_tensor("emb", [K, C], F32, kind="ExternalInput")
    zq = nc.dram_tensor("zq", [C, TOK_PER_CORE], F32, kind="ExternalOutput")
    lossp = nc.dram_tensor("lossp", [128, 2], F32, kind="ExternalOutput")

    z_view = z.ap().rearrange("(co p) n -> p co n", p=128)
    zq_view = zq.ap().rearrange("(co p) n -> p co n", p=128)

    with tile.TileContext(nc) as tc:
        with (
            tc.tile_pool(name="const", bufs=1) as const,
            tc.tile_pool(name="zg", bufs=2) as zg_pool,
            tc.tile_pool(name="outg", bufs=2) as outg_pool,
            tc.tile_pool(name="scores", bufs=3) as scores_pool,
            tc.tile_pool(name="gather", bufs=4) as gather_pool,
            tc.tile_pool(name="idx", bufs=4) as idx_pool,
            tc.tile_pool(name="scr", bufs=2) as scr_pool,
            tc.tile_pool(name="psc", bufs=2, space="PSUM") as psum_cross,
            tc.tile_pool(name="pst", bufs=4, space="PSUM") as psum_tr,
        ):
            ident = const.tile([128, 128], F32)
            make_identity(nc, ident[:])

            # persistent loss accumulators
            maxbuf = const.tile([128, N_TILES * 8], F32)   # top-8 per tile
            z2acc = const.tile([128, N_GROUPS], F32)
            lossout = const.tile([128, 2], F32)

            embT = const.tile([128, 2, K], F32)  # [c_in_chunk, c_chunk, k]
            esq_bcast = const.tile([128, K], F32)  # -|e_k|^2/2, same all parts

            with tc.tile_pool(name="setup", bufs=1) as setup:
                emb_sb = setup.tile([128, 8, C], F32)  # [k%128, k//128, c]
                nc.sync.dma_start(
                    emb_sb[:], emb.ap().rearrange("(ko p) c -> p ko c", p=128)
                )
                for ko in range(8):
                    for co in range(2):
                        pt = psum_tr.tile([128, 128], F32, tag="tr")
                        nc.tensor.transpose(
                            pt[:],
                            emb_sb[:, ko, co * 128 : (co + 1) * 128],
                            ident[:],
                        )
                        nc.scalar.copy(
                            embT[:, co, ko * 128 : (ko + 1) * 128], pt[:]
                        )

                # esq_bcast[p, k] = -0.5 * sum_c embT[c, k]^2 for every p:
                # all-ones lhsT makes each PSUM partition the full column sum.
                embT_sq = setup.tile([128, 2, K], F32)
                nc.vector.tensor_mul(embT_sq[:], embT[:], embT[:])
                ones_t = setup.tile([128, 128], F32)
                nc.vector.memset(ones_t[:], 1.0)
                esq_psum = psum_cross.tile([128, K], F32, tag="cross")
                for h in range(2):
                    for co in range(2):
                        nc.tensor.matmul(
                            esq_psum[:, h * 512 : (h + 1) * 512],
                            lhsT=ones_t[:],
                            rhs=embT_sq[:, co, h * 512 : (h + 1) * 512],
                            start=(co == 0),
                            stop=(co == 1),
                        )
                nc.vector.tensor_scalar_mul(esq_bcast[:], esq_psum[:], -0.5)

            for g in range(N_GROUPS):
                n0 = g * GROUP
                zg = zg_pool.tile([128, 2, GROUP], F32)
                nc.sync.dma_start(zg[:], z_view[:, :, n0 : n0 + GROUP])
                outg = outg_pool.tile([128, 2, GROUP], F32)

                # sum(z^2) for the loss
                scr = scr_pool.tile([128, 2, GROUP], F32)
                nc.scalar.activation(
                    scr[:],
                    zg[:],
                    mybir.ActivationFunctionType.Square,
                    accum_out=z2acc[:, g : g + 1],
                )

                for i in range(TILES_PER_GROUP):
                    t0 = i * 128
                    s = g * TILES_PER_GROUP + i  # global tile index
                    cross = psum_cross.tile([128, K], F32, tag="cross")
                    for co in range(2):
                        lhsT = zg[:, co, t0 : t0 + 128]
                        for h in range(2):
                            nc.tensor.matmul(
                                cross[:, h * 512 : (h + 1) * 512],
                                lhsT=lhsT,
                                rhs=embT[:, co, h * 512 : (h + 1) * 512],
                                start=(co == 0),
                                stop=(co == 1),
                            )
                    scores = scores_pool.tile([128, K], F32)
                    nc.vector.tensor_add(scores[:], cross[:], esq_bcast[:])

                    mx8 = maxbuf[:, s * 8 : (s + 1) * 8]
                    nc.vector.max(out=mx8, in_=scores[:])
                    idx = idx_pool.tile([128, 8], mybir.dt.uint32)
                    nc.vector.max_index(out=idx[:], in_max=mx8, in_values=scores[:])

                    gath = gather_pool.tile([128, C], F32)
                    nc.gpsimd.indirect_dma_start(
                        out=gath[:],
                        out_offset=None,
                        in_=emb.ap(),
                        in_offset=bass.IndirectOffsetOnAxis(
                            ap=idx[:, 0:1], axis=0
                        ),
                    )
                    for co in range(2):
                        ptr = psum_tr.tile([128, 128], F32, tag="tr")
                        nc.tensor.transpose(
                            ptr[:], gath[:, co * 128 : (co + 1) * 128], ident[:]
                        )
                        nc.scalar.copy(outg[:, co, t0 : t0 + 128], ptr[:])

                nc.sync.dma_start(zq_view[:, :, n0 : n0 + GROUP], outg[:])

            # loss partials: col0 = sum(z^2), col1 = sum(max score)
            nc.vector.reduce_sum(
                out=lossout[:, 0:1], in_=z2acc[:], axis=mybir.AxisListType.X
            )
            mxcol = maxbuf[:].rearrange("p (i e) -> p e i", e=8)[:, 0, :]
            nc.vector.reduce_sum(
                out=lossout[:, 1:2], in_=mxcol, axis=mybir.AxisListType.X
            )
            nc.sync.dma_start(lossp.ap(), lossout[:])

    _split_multi_waits(nc)
    return nc


_NC_CACHE = None


def _get_nc():
    global _NC_CACHE
    if _NC_CACHE is None:
        _NC_CACHE = build()
    return _NC_CACHE


def _shard_inputs(z_e, emb):
    zr = np.ascontiguousarray(z_e.reshape(B, C, DHW))
    in_maps = []
    for core in range(N_CORES):
        b, half = core // 2, core % 2
        shard = np.ascontiguousarray(
            zr[b, :, half * TOK_PER_CORE : (half + 1) * TOK_PER_CORE]
        )
        in_maps.append({"z": shard, "emb": np.ascontiguousarray(emb)})
    return in_maps


def _assemble(results):
    zq_full = np.empty((B, C, DHW), dtype=np.float32)
    total_z2 = 0.0
    total_mx = 0.0
    for core in range(N_CORES):
        b, half = core // 2, core % 2
        zq_full[b, :, half * TOK_PER_CORE : (half + 1) * TOK_PER_CORE] = results[
            core
        ]["zq"]
        lp = results[core]["lossp"].astype(np.float64)
        total_z2 += lp[:, 0].sum()
        total_mx += lp[:, 1].sum()
    loss_sum = total_z2 - 2.0 * total_mx
    mean_sq = loss_sum / float(B * DHW * C)
    vq_loss = np.float32(mean_sq + 0.25 * mean_sq)
    return zq_full.reshape(B, C, 32, 32, 32), vq_loss


def run_spmd(z_e, emb, **kwargs):
    nc = _get_nc()
    in_maps = _shard_inputs(np.asarray(z_e), np.asarray(emb))
    res = bass_utils.run_bass_kernel_spmd(
        nc, in_maps, core_ids=list(range(N_CORES)), **kwargs
    )
    return res


def kernel(z_e, emb):
    res = run_spmd(z_e, emb)
    return _assemble(res.results)


# revision 18
# speedup vs baseline: 1.0031x; 1.0031x over previous
"""VQ codebook quantization kernel for Trainium2 (8 NeuronCores, SPMD).

Problem: z_e [4, 256, 32, 32, 32] f32, emb [1024, 256] f32.
  tokens = z_e reshaped to [B, N=32768, C=256]; for each token find the
  nearest codebook row (squared L2), output the gathered embeddings in the
  original [B, C, D, H, W] layout plus the scalar VQ loss.

Sharding: data-parallel over the 131072 tokens -> 8 shards of 16384 tokens
(core i handles batch b = i//2, token half i%2). The [1024, 256] codebook is
replicated. Each core returns its z_q shard plus per-partition loss partials;
the host reassembles the full tensor and reduces the loss scalar.

Per-core algorithm (all fp32):
  score[t, k] = z_t . e_k - |e_k|^2/2   (argmax_k score == argmin_k dist^2)
  - cross term via PE matmuls: lhsT = z slice [c,t] (natural DRAM layout),
    rhs = embT (one-time on-chip transpose of emb)
  - bias -|e_k|^2/2 precomputed broadcast across partitions (one-time),
    added during the PSUM->SBUF move (DVE)
  - argmax over k via DVE max8 + max_index
  - gather emb[idx] via indirect DMA (one row per partition), transpose the
    gathered [t, c] tile back to [c, t] on the PE
  - loss uses sum((z - e*)^2) = sum(z^2) - 2 * sum(max score): no per-element
    diff needed.  sum(z^2) via ACT Square-accumulate, sum(max) from the max8
    results.
"""

import numpy as np

import concourse.bass as bass
import concourse.mybir as mybir
import concourse.tile as tile
from concourse import bass_utils

F32 = mybir.dt.float32
F32R = mybir.dt.float32r
F16 = mybir.dt.float16

B, C, DHW = 4, 256, 32 * 32 * 32
K = 1024
N_CORES = 8
TOK_PER_CORE = (B * DHW) // N_CORES  # 16384
GROUP = 1024                         # tokens per DMA group
N_GROUPS = TOK_PER_CORE // GROUP     # 16
TILES_PER_GROUP = GROUP // 128       # 8
N_TILES = N_GROUPS * TILES_PER_GROUP # 128


def _split_multi_waits(nc):
    """This container's walrus rejects more than one sync wait per
    instruction.  Hoist excess waits onto wait-only EventSemaphore carriers
    inserted immediately before the offending instruction on the same
    engine's stream (semantically identical: the engine blocks on the
    carrier first)."""
    n_carrier = 0
    for bb in nc.m.functions[0].blocks:
        insts = list(bb.instructions)
        if not any(
            i.sync_info is not None and i.sync_info.on_wait and len(i.sync_info.on_wait) > 1
            for i in insts
        ):
            continue
        newlist = []
        for ins in insts:
            si = ins.sync_info
            if si is not None and si.on_wait and len(si.on_wait) > 1:
                waits = list(si.on_wait)
                for w in waits[:-1]:
                    ev = mybir.InstEventSemaphore(
                        name=f"wsplit-{n_carrier}", ins=[], outs=[]
                    )
                    n_carrier += 1
                    ev.engine = ins.engine
                    ev.sync_info = mybir.SyncInfo(on_wait=[w], on_update=[])
                    newlist.append(ev)
                ins.sync_info = mybir.SyncInfo(
                    on_wait=[waits[-1]], on_update=list(si.on_update or [])
                )
            newlist.append(ins)
        bb.instructions = newlist


def build():
    from concourse.masks import make_identity

    nc = bass.Bass("TRN2", target_bir_lowering=False, debug=False)
    z = nc.dram_tensor("z", [C, TOK_PER_CORE], F32, kind="ExternalInput")
    emb = nc.dram_tensor("emb", [K, C], F32, kind="ExternalInput")
    zq = nc.dram_tensor("zq", [C, TOK_PER_CORE], F32, kind="ExternalOutput")
    lossp = nc.dram_tensor("lossp", [128, 2], F32, kind="ExternalOutput")

    z_view = z.ap().rearrange("(co p) n -> p co n", p=128)
    zq_view = zq.ap().rearrange("(co p) n -> p co n", p=128)

    with tile.TileContext(nc) as tc:
        with (
            tc.tile_pool(name="const", bufs=1) as const,
            tc.tile_pool(name="zg", bufs=2) as zg_pool,
            tc.tile_pool(name="outg", bufs=2) as outg_pool,
            tc.tile_pool(name="scores", bufs=3) as scores_pool,
            tc.tile_pool(name="gather", bufs=4) as gather_pool,
            tc.tile_pool(name="idx", bufs=4) as idx_pool,
            tc.tile_pool(name="scr", bufs=2) as scr_pool,
            tc.tile_pool(name="zgh", bufs=2) as zgh_pool,
            tc.tile_pool(name="zgl", bufs=2) as zgl_pool,
            tc.tile_pool(name="psc", bufs=2, space="PSUM") as psum_cross,
            tc.tile_pool(name="pst", bufs=4, space="PSUM") as psum_tr,
        ):
            ident = const.tile([128, 128], F32)
            make_identity(nc, ident[:])

            # persistent loss accumulators
            maxbuf = const.tile([128, N_TILES * 8], F32)   # top-8 per tile
            z2acc = const.tile([128, N_GROUPS], F32)
            lossout = const.tile([128, 2], F32)

            embT = const.tile([128, 2, K], F32)  # [c_in_chunk, c_chunk, k]
            esq_bcast = const.tile([128, K], F32)  # -|e_k|^2/2, same all parts
            # fp16 hi/lo split of embT for the 3-term exact-enough matmul:
            # z.e = zh.eh + zh.el + zl.eh (+ zl.el, negligible at fp16)
            embT_h = const.tile([128, 2, K], F16)
            embT_l = const.tile([128, 2, K], F16)

            with tc.tile_pool(name="setup", bufs=1) as setup:
                emb_sb = setup.tile([128, 8, C], F32)  # [k%128, k//128, c]
                nc.sync.dma_start(
                    emb_sb[:], emb.ap().rearrange("(ko p) c -> p ko c", p=128)
                )
                for ko in range(8):
                    for co in range(2):
                        pt = psum_tr.tile([128, 128], F32, tag="tr")
                        nc.tensor.transpose(
                            pt[:],
                            emb_sb[:, ko, co * 128 : (co + 1) * 128],
                            ident[:],
                        )
                        nc.scalar.copy(
                            embT[:, co, ko * 128 : (ko + 1) * 128], pt[:]
                        )

                # esq_bcast[p, k] = -0.5 * sum_c embT[c, k]^2 for every p:
                # all-ones lhsT makes each PSUM partition the full column sum.
                embT_sq = setup.tile([128, 2, K], F32)
                nc.vector.tensor_mul(embT_sq[:], embT[:], embT[:])
                ones_t = setup.tile([128, 128], F32)
                nc.vector.memset(ones_t[:], 1.0)
                esq_psum = psum_cross.tile([128, K], F32, tag="cross")
                for h in range(2):
                    for co in range(2):
                        nc.tensor.matmul(
                            esq_psum[:, h * 512 : (h + 1) * 512],
                            lhsT=ones_t[:],
                            rhs=embT_sq[:, co, h * 512 : (h + 1) * 512],
                            start=(co == 0),
                            stop=(co == 1),
                        )
                nc.vector.tensor_scalar_mul(esq_bcast[:], esq_psum[:], -0.5)

                nc.vector.tensor_copy(embT_h[:], embT[:])
                nc.vector.tensor_sub(embT_l[:], embT[:], embT_h[:])

            for g in range(N_GROUPS):
                n0 = g * GROUP
                zg = zg_pool.tile([128, 2, GROUP], F32)
                nc.sync.dma_start(zg[:], z_view[:, :, n0 : n0 + GROUP])
                outg = outg_pool.tile([128, 2, GROUP], F32)

                # sum(z^2) for the loss
                scr = scr_pool.tile([128, 2, GROUP], F32)
                nc.scalar.activation(
                    scr[:],
                    zg[:],
                    mybir.ActivationFunctionType.Square,
                    accum_out=z2acc[:, g : g + 1],
                )

                # fp16 hi/lo split of this group's z
                zgh = zgh_pool.tile([128, 2, GROUP], F16)
                nc.scalar.copy(zgh[:], zg[:])
                zgl = zgl_pool.tile([128, 2, GROUP], F16)
                nc.vector.tensor_sub(zgl[:], zg[:], zgh[:])

                for i in range(TILES_PER_GROUP):
                    t0 = i * 128
                    s = g * TILES_PER_GROUP + i  # global tile index
                    cross = psum_cross.tile([128, K], F32, tag="cross")
                    terms = [
                        (zgh, embT_h),
                        (zgh, embT_l),
                        (zgl, embT_h),
                    ]
                    for h in range(2):
                        nt = 0
                        for zt, et in terms:
                            for co in range(2):
                                nc.tensor.matmul(
                                    cross[:, h * 512 : (h + 1) * 512],
                                    lhsT=zt[:, co, t0 : t0 + 128],
                                    rhs=et[:, co, h * 512 : (h + 1) * 512],
                                    start=(nt == 0),
                                    stop=(nt == 5),
                                )
                                nt += 1
                    scores = scores_pool.tile([128, K], F32)
                    nc.vector.tensor_add(scores[:], cross[:], esq_bcast[:])

                    mx8 = maxbuf[:, s * 8 : (s + 1) * 8]
                    nc.vector.max(out=mx8, in_=scores[:])
                    idx = idx_pool.tile([128, 8], mybir.dt.uint32)
                    nc.vector.max_index(out=idx[:], in_max=mx8, in_values=scores[:])

                    gath = gather_pool.tile([128, C], F32)
                    nc.gpsimd.indirect_dma_start(
                        out=gath[:],
                        out_offset=None,
                        in_=emb.ap(),
                        in_offset=bass.IndirectOffsetOnAxis(
                            ap=idx[:, 0:1], axis=0
                        ),
                    )
                    for co in range(2):
                        ptr = psum_tr.tile([128, 128], F32, tag="tr")
                        nc.tensor.transpose(
                            ptr[:], gath[:, co * 128 : (co + 1) * 128], ident[:]
                        )
                        nc.scalar.copy(outg[:, co, t0 : t0 + 128], ptr[:])

                nc.sync.dma_start(zq_view[:, :, n0 : n0 + GROUP], outg[:])

            # loss partials: col0 = sum(z^2), col1 = sum(max score)
            nc.vector.reduce_sum(
                out=lossout[:, 0:1], in_=z2acc[:], axis=mybir.AxisListType.X
            )
            mxcol = maxbuf[:].rearrange("p (i e) -> p e i", e=8)[:, 0, :]
            nc.vector.reduce_sum(
                out=lossout[:, 1:2], in_=mxcol, axis=mybir.AxisListType.X
            )
            nc.sync.dma_start(lossp.ap(), lossout[:])

    _split_multi_waits(nc)
    return nc


_NC_CACHE = None


def _get_nc():
    global _NC_CACHE
    if _NC_CACHE is None:
        _NC_CACHE = build()
    return _NC_CACHE


def _shard_inputs(z_e, emb):
    zr = np.ascontiguousarray(z_e.reshape(B, C, DHW))
    in_maps = []
    for core in range(N_CORES):
        b, half = core // 2, core % 2
        shard = np.ascontiguousarray(
            zr[b, :, half * TOK_PER_CORE : (half + 1) * TOK_PER_CORE]
        )
        in_maps.append({"z": shard, "emb": np.ascontiguousarray(emb)})
    return in_maps


def _assemble(results):
    zq_full = np.empty((B, C, DHW), dtype=np.float32)
    total_z2 = 0.0
    total_mx = 0.0
    for core in range(N_CORES):
        b, half = core // 2, core % 2
        zq_full[b, :, half * TOK_PER_CORE : (half + 1) * TOK_PER_CORE] = results[
            core
        ]["zq"]
        lp = results[core]["lossp"].astype(np.float64)
        total_z2 += lp[:, 0].sum()
        total_mx += lp[:, 1].sum()
    loss_sum = total_z2 - 2.0 * total_mx
    mean_sq = loss_sum / float(B * DHW * C)
    vq_loss = np.float32(mean_sq + 0.25 * mean_sq)
    return zq_full.reshape(B, C, 32, 32, 32), vq_loss


def run_spmd(z_e, emb, **kwargs):
    nc = _get_nc()
    in_maps = _shard_inputs(np.asarray(z_e), np.asarray(emb))
    res = bass_utils.run_bass_kernel_spmd(
        nc, in_maps, core_ids=list(range(N_CORES)), **kwargs
    )
    return res


def kernel(z_e, emb):
    res = run_spmd(z_e, emb)
    return _assemble(res.results)
